# revision 9
# baseline (speedup 1.0000x reference)
"""Causal self-attention Trainium2 kernel (v2: bf16 + col-tiled PV/denom).

Problem: x:[2,2048,1024] f32, w_qkv:[1024,16,192], w_out:[16,64,1024].
  qkv = einsum('bse,ehd->bshd', x, w_qkv); q,k,v = split(qkv, 3, -1)
  att = causal_softmax(q k^T / sqrt(64)) v ;  y = einsum('bshd,hde->bse', att, w_out)

Sharding: 8 cores = batch(2) x head-group(4 heads each).  Each core computes a
partial y (its 4 heads' contribution) for its batch; the host sums the 4
partials per batch.  All device compute in bf16 (tolerance 2e-2; bf16 path
measures ~5e-3), PSUM accumulation fp32.

Per-core dataflow, software-pipelined over q-blocks j (ScalarE exp is the
~58us wall; everything else hides under it):
  xT [1024,2048] bf16 loaded in s-chunk-major order so pass A starts after
    1MB instead of the full 4MB.
  pass A group(n): qkT[4x128, 512] for s-chunk n   (m order k01,q01,k23,q23)
  pass B group(g): v[s-tile t, 256] for t in 4g..4g+3
  attention j: per k-tile i: logitsT[k,q] = kT^T qT (2 head-pairs, K=64 row
    groups 0-63/64-127 run concurrently); pT = exp(0.125*logitsT) on ScalarE;
    tri-mask on diagonal tiles; PV pair col-tiled: h-even -> psum rows 0:64,
    h-odd -> rows 64:128 of ONE bank (tile_position (0,0)/(0,64) concurrent);
    denominators: 4 x M=1 matmuls (ones lhsT) col-tiled at psum rows 0/32/64/96.
    A/B/Y matmul groups for j+1 / j-1 are interleaved as fillers so the PE
    queue never starves ScalarE.
  normalize j: DVE recip of denom rows; GpSimd partition_broadcast spreads
    recip rows across partitions (idle engine); DVE mul -> at_sb bf16.
  out-proj (filler Y(j)): y[s,e] psum = attnT^T w_o; per-s-tile 256KB stores.
PSUM banks: psL 2x2 + psD 1 + ps1 3 = 8.  Shared-bank accumulators use
memset-0 + start=False everywhere (safe under any has_written semantics).
"""

import numpy as np

B, S, E = 2, 2048, 1024
H, D = 16, 64
HC = 4          # heads per core
NCORES = 8
SB = S // 512   # 4 q-blocks of 512
KT = S // 128   # 16 k-tiles of 128
ET = E // 128   # 8 e-tiles

_cached = {}


def _build_program(unroll=1):
    import concourse.bass as bass  # noqa: F401
    import concourse.tile as tile
    from concourse import bacc, mybir
    from contextlib import ExitStack

    f32 = mybir.dt.float32
    bf16 = mybir.dt.bfloat16
    Exp = mybir.ActivationFunctionType.Exp

    nc = bacc.Bacc("TRN2", target_bir_lowering=False, debug=False)
    xT_d = nc.declare_dram_parameter("xT", [E, S], bf16, isOutput=False)
    wqk_d = nc.declare_dram_parameter("w_qk", [E, 512], bf16, isOutput=False)
    wv_d = nc.declare_dram_parameter("w_v", [E, 256], bf16, isOutput=False)
    wo_d = nc.declare_dram_parameter("w_o", [256, E], bf16, isOutput=False)
    tri_d = nc.declare_dram_parameter("tri", [128, 128], bf16, isOutput=False)
    blk_d = nc.declare_dram_parameter("blk", [2, 128], bf16, isOutput=False)
    y_d = nc.declare_dram_parameter("y", [S, E], bf16, isOutput=True)

    xT_r = xT_d.rearrange("(t p) s -> p t s", p=128)

    with tile.TileContext(nc) as tc:
      for _rep in range(unroll):
        with ExitStack() as ctx:
            persist = ctx.enter_context(tc.tile_pool(name="persist", bufs=1))

            xT_sb = persist.tile([128, ET, S], bf16, tag="xT")
            wqk_sb = persist.tile([128, ET, 512], bf16, tag="wqk")
            wv_sb = persist.tile([128, ET, 256], bf16, tag="wv")
            wo_sb = persist.tile([128, 2, E], bf16, tag="wo")
            tri_sb = persist.tile([128, 128], bf16, tag="tri")
            onesd = persist.tile([128, 1], bf16, tag="onesd")
            blk_sb = persist.tile([128, 128], bf16, tag="blk")
            qk_sb = persist.tile([128, 4, S], bf16, tag="qk")
            v_sb = persist.tile([128, KT, 256], bf16, tag="v")
            at_sb = persist.tile([128, 2, S], bf16, tag="attnT")

            nc.sync.dma_start(
                out=wqk_sb, in_=wqk_d.rearrange("(t p) m -> p t m", p=128))
            nc.sync.dma_start(
                out=wv_sb, in_=wv_d.rearrange("(t p) m -> p t m", p=128))
            nc.sync.dma_start(
                out=wo_sb, in_=wo_d.rearrange("(t p) m -> p t m", p=128))
            nc.sync.dma_start(out=tri_sb, in_=tri_d[:, :])
            nc.sync.dma_start(out=blk_sb[0:2, :], in_=blk_d[:, :])
            nc.sync.dma_start(out=blk_sb[64:66, :], in_=blk_d[:, :])
            # s-chunk-major: pass A group(n) only needs s-columns
            # [512n, 512n+512) of every e-tile, so it can start after 1MB.
            for s in range(SB):
                for k in range(ET):
                    nc.sync.dma_start(
                        out=xT_sb[:, k, s * 512:(s + 1) * 512],
                        in_=xT_r[:, k, s * 512:(s + 1) * 512])
            nc.vector.memset(onesd, 1.0)

            with tc.tile_pool(name="psL", bufs=2, space="PSUM") as psL, \
                 tc.tile_pool(name="psD", bufs=1, space="PSUM") as psDp, \
                 tc.tile_pool(name="ps1", bufs=3, space="PSUM") as ps1, \
                 tc.tile_pool(name="pt", bufs=3) as ptp, \
                 tc.tile_pool(name="nrm", bufs=2) as nrm, \
                 tc.tile_pool(name="bcp", bufs=2) as bcp, \
                 tc.tile_pool(name="ytp", bufs=2) as ytp:

                def a_unit(n, m):
                    # qkT[:, m, 512n:512n+512] = w_qk[:, m-tile]^T @ xT chunk
                    ps = ps1.tile([128, 512], f32, tag="ps1", name=f"A_{n}_{m}")
                    for k in range(ET):
                        nc.tensor.matmul(
                            ps,
                            wqk_sb[:, k, m * 128:(m + 1) * 128],
                            xT_sb[:, k, n * 512:(n + 1) * 512],
                            start=(k == 0),
                            stop=(k == ET - 1),
                        )
                    nc.vector.tensor_copy(
                        qk_sb[:, m, n * 512:(n + 1) * 512], ps)

                def b_unit(t):
                    # v_sb[:, t, :] = x[s-tile t] @ w_v
                    ps = ps1.tile([128, 512], f32, tag="ps1", name=f"B_{t}")
                    for k in range(ET):
                        nc.tensor.matmul(
                            ps[:, 0:256],
                            xT_sb[:, k, t * 128:(t + 1) * 128],
                            wv_sb[:, k, :],
                            start=(k == 0),
                            stop=(k == ET - 1),
                        )
                    nc.vector.tensor_copy(v_sb[:, t, :], ps[:, 0:256])

                def y_unit(t):
                    # y[s-tile t, :] = attnT[:, t]^T @ w_o  (256KB store)
                    yt = ytp.tile([128, E], bf16, tag="yt", name=f"yt_{t}")
                    for n in range(2):
                        ps = ps1.tile([128, 512], f32, tag="ps1",
                                      name=f"Y_{t}_{n}")
                        for c in range(2):
                            nc.tensor.matmul(
                                ps,
                                at_sb[:, c, t * 128:(t + 1) * 128],
                                wo_sb[:, c, n * 512:(n + 1) * 512],
                                start=(c == 0),
                                stop=(c == 1),
                            )
                        nc.vector.tensor_copy(yt[:, n * 512:(n + 1) * 512], ps)
                    nc.sync.dma_start(out=y_d[t * 128:(t + 1) * 128, :], in_=yt)

                # prologue: everything attention j=0 needs
                for m in (1, 0, 3, 2):
                    a_unit(0, m)
                for t in range(4):
                    b_unit(t)

                for j in range(SB):
                    # fillers to interleave into this j's i-loop: projections
                    # for j+1 and out-proj for j-1 (deps are already met).
                    fillers = []
                    if j + 1 < SB:
                        for m in (1, 0, 3, 2):
                            fillers.append((a_unit, (j + 1, m)))
                        for t in range(4 * (j + 1), 4 * (j + 1) + 4):
                            fillers.append((b_unit, (t,)))
                    if j >= 1:
                        for t in range(4 * (j - 1), 4 * (j - 1) + 4):
                            fillers.append((y_unit, (t,)))

                    oa01 = ps1.tile([128, 512], f32, tag="ps1", name=f"oa01_{j}")
                    oa23 = ps1.tile([128, 512], f32, tag="ps1", name=f"oa23_{j}")
                    psD = psDp.tile([128, 512], f32, tag="psD", name=f"psD_{j}")
                    nc.vector.memset(oa01, 0.0)
                    nc.vector.memset(oa23, 0.0)
                    nc.vector.memset(psD, 1.0)  # keep recip finite on unused rows
                    for h in range(HC):
                        nc.vector.memset(psD[32 * h:32 * h + 1, :], 0.0)

                    nk = 4 * j + 4
                    for i in range(nk):
                        off = max(0, (i - 4 * j) * 128)
                        q0 = j * 512 + off
                        lgs, pts = [], []
                        for p in range(2):  # head pairs (0,1), (2,3)
                            lg = psL.tile([128, 2, 512], f32, tag="lg",
                                          name=f"lg_{j}_{i}_{p}")
                            lgs.append(lg)
                            for sub in range(2):
                                r0 = sub * 64
                                nc.tensor.matmul(
                                    lg[:, sub, off:512],
                                    qk_sb[r0:r0 + 64, 2 * p + 1, i * 128:(i + 1) * 128],
                                    qk_sb[r0:r0 + 64, 2 * p, q0:(j + 1) * 512],
                                    start=True,
                                    stop=True,
                                )
                        for p in range(2):
                            pt = ptp.tile([128, 2, 512], bf16, tag="pt",
                                          name=f"pt_{j}_{i}_{p}")
                            pts.append(pt)
                            nc.scalar.activation(
                                pt[:, :, off:512], lgs[p][:, :, off:512],
                                Exp, scale=0.125)
                            if i >= 4 * j:  # diagonal-crossing k-tile
                                for sub in range(2):
                                    nc.vector.tensor_mul(
                                        pt[:, sub, off:off + 128],
                                        pt[:, sub, off:off + 128],
                                        tri_sb,
                                    )
                        # PV: col-tiled pair per psum bank; h-even -> rows
                        # 0:64 (tile (0,0)), h-odd -> rows 64:128 ((0,64)).
                        for p, oa in ((0, oa01), (1, oa23)):
                            for sub in range(2):
                                h = 2 * p + sub
                                nc.tensor.matmul(
                                    oa[64 * sub:64 * sub + 64, off:512],
                                    v_sb[:, i, h * 64:(h + 1) * 64],
                                    pts[p][:, sub, off:512],
                                    start=False,
                                    stop=(i == nk - 1),
                                    skip_group_check=True,
                                )
                        # denominators: M=1 matmuls col-tiled at rows 32h
                        for h in range(HC):
                            p, sub = h // 2, h % 2
                            nc.tensor.matmul(
                                psD[32 * h:32 * h + 1, off:512],
                                onesd,
                                pts[p][:, sub, off:512],
                                start=False,
                                stop=(i == nk - 1),
                                skip_group_check=True,
                                tile_position=(0, 32 * h),
                            )
                        # interleave filler matmul groups to keep PE queue
                        # ahead of ScalarE without starving it at j bounds
                        nfill = -(-len(fillers) // (nk - i))  # ceil
                        for fn, args in fillers[:nfill]:
                            fn(*args)
                        fillers = fillers[nfill:]

                    # normalize: recip denom rows (bf16), DMA-gather the 4
                    # rows to adjacent partitions, broadcast across
                    # partitions with a K=2 block-ones matmul, multiply
                    sdb = nrm.tile([97, 512], bf16, tag="sd", name=f"sd_{j}")
                    with nc.allow_low_precision(reason="softmax recip bf16"):
                        nc.vector.reciprocal(sdb, psD[0:97, :])
                    rd = nrm.tile([128, 512], bf16, tag="rd", name=f"rd_{j}")
                    nc.sync.dma_start(out=rd[0:1, :], in_=sdb[0:1, :])
                    nc.sync.dma_start(out=rd[1:2, :], in_=sdb[32:33, :])
                    nc.sync.dma_start(out=rd[64:65, :], in_=sdb[64:65, :])
                    nc.sync.dma_start(out=rd[65:66, :], in_=sdb[96:97, :])
                    for pr in range(2):
                        r0 = 64 * pr
                        bcps = ps1.tile([128, 512], f32, tag="ps1",
                                        name=f"bc_{j}_{pr}")
                        nc.tensor.matmul(bcps, blk_sb[r0:r0 + 2, :],
                                         rd[r0:r0 + 2, :],
                                         start=True, stop=True)
                        bcs = bcp.tile([128, 512], f32, tag="bc",
                                       name=f"bcs_{j}_{pr}")
                        nc.vector.tensor_copy(bcs, bcps)
                        oa = oa01 if pr == 0 else oa23
                        nc.vector.tensor_mul(
                            at_sb[:, pr, j * 512:(j + 1) * 512], oa, bcs)

                # epilogue: out-proj for the last q-block
                for t in range(4 * (SB - 1), 4 * SB):
                    y_unit(t)
    nc.compile()
    return nc


def _prep_inputs(x, w_qkv, w_out):
    """Build the 8 per-core input maps. Core c = batch(c//4), head-group(c%4)."""
    import ml_dtypes
    bf16 = ml_dtypes.bfloat16
    tri = np.triu(np.ones((128, 128), dtype=np.float32))  # [k,q] keep k<=q
    xT = [np.ascontiguousarray(x[b].T).astype(bf16) for b in range(B)]
    in_maps = []
    for c in range(NCORES):
        b, g = c // 4, c % 4
        hs = [g * HC + l for l in range(HC)]
        # w_qk [1024, 512]: m-tiles = [q_h0|q_h1], [k_h0|k_h1], [q_h2|q_h3], [k_h2|k_h3]
        cols = []
        for pair in range(2):
            h0, h1 = hs[2 * pair], hs[2 * pair + 1]
            cols.append(np.concatenate([w_qkv[:, h0, 0:64], w_qkv[:, h1, 0:64]], axis=1))
            cols.append(np.concatenate([w_qkv[:, h0, 64:128], w_qkv[:, h1, 64:128]], axis=1))
        w_qk = np.ascontiguousarray(np.concatenate(cols, axis=1)).astype(bf16)
        w_v = np.ascontiguousarray(
            np.concatenate([w_qkv[:, h, 128:192] for h in hs], axis=1)).astype(bf16)
        w_o = np.ascontiguousarray(
            w_out[hs[0]:hs[0] + HC].reshape(HC * D, E)).astype(bf16)
        blk = np.zeros((2, 128), dtype=bf16)
        blk[0, 0:64] = 1
        blk[1, 64:128] = 1
        in_maps.append({"xT": xT[b], "w_qk": w_qk, "w_v": w_v, "w_o": w_o,
                        "tri": tri.astype(bf16), "blk": blk})
    return in_maps


def kernel(x, w_qkv, w_out):
    from concourse.bass_utils import run_bass_kernel_spmd

    if "nc" not in _cached:
        _cached["nc"] = _build_program()
    nc = _cached["nc"]
    in_maps = _prep_inputs(np.asarray(x), np.asarray(w_qkv), np.asarray(w_out))
    res = run_bass_kernel_spmd(nc, in_maps, list(range(NCORES))).results
    y = np.zeros((B, S, E), dtype=np.float32)
    for c in range(NCORES):
        y[c // 4] += np.asarray(res[c]["y"], dtype=np.float32)
    return y
